# revision 4
# baseline (speedup 1.0000x reference)
"""Distributed GATv2 GNN kernel for trn2 (8 NeuronCores).

Sharding: nodes are degree-sorted and striped across 8 cores (graph
partition by destination). Each core processes the incoming-edge segments
of its nodes in blocks of 128 dst lanes with a uniform per-block padded
degree schedule K_sched. Per layer:
  - every core computes the full xl = h @ Wl table (node-major, DRAM),
  - per dst block: dma_gather of xl[src] rows, GATv2 attention + softmax +
    weighted sum on DVE/ACT, PE transpose into a feature-major local slab,
  - AllGather of the pre-BN output slab, then global BN stats + ReLU
    (+ residual) computed redundantly on every core.
Weights are replicated; indices/masks are host-precomputed constants.
"""
import numpy as np
from dataclasses import dataclass

import concourse.bass as bass
import concourse.bacc as bacc
import concourse.tile as tile
import concourse.mybir as mybir

AF = mybir.ActivationFunctionType
ALU = mybir.AluOpType
FP32 = mybir.dt.float32
I16 = mybir.dt.int16

SLOPE = 0.2
EPS = 1e-5
NEG = -1.0e30


@dataclass
class Cfg:
    ncores: int = 8
    blocks: int = 20               # dst blocks per core
    real_per_core: int = 2500      # real nodes per core
    nlayers: int = 20              # total GAT layers (first + mids + final)
    K_sched: tuple = ()            # per-block padded degree (shared by cores)
    f_in: int = 17
    stage: int = 4
    bstage: int = 7
    gather_layers: tuple = tuple(range(32))
    dump_layer: int = -1           # if >=0: dump h_fold after this layer and stop

    @property
    def slots(self):
        return self.blocks * 128

    @property
    def npad(self):
        return self.ncores * self.slots

    @property
    def half2(self):
        return self.npad // 2

    @property
    def sumk(self):
        return int(sum(self.K_sched))

    def width(self, l):           # H*D of layer l
        return 128 if l < self.nlayers - 1 else 64

    def kcontract(self, l):       # matmul contraction dim
        return 32 if l == 0 else 64


def build_kernel(cfg: Cfg):
    NC = cfg.ncores
    SLOTS = cfg.slots
    NPAD = cfg.npad
    HALF2 = cfg.half2
    L = cfg.nlayers
    SUMK = cfg.sumk
    KMAX = int(max(cfg.K_sched))
    N_REAL = NC * cfg.real_per_core
    X = mybir.AxisListType.X

    nc = bacc.Bacc("TRN2", target_bir_lowering=False, debug=False, num_devices=NC)

    # ---------------- DRAM I/O ----------------
    idx_d = nc.dram_tensor("idx", [128, 8 * SUMK], I16, kind="ExternalInput")
    mask_d = nc.dram_tensor("mask", [128, SUMK], FP32, kind="ExternalInput")
    xTf_d = nc.dram_tensor("xTf", [64, HALF2], FP32, kind="ExternalInput")
    xloc_d = nc.dram_tensor("xloc", [32, SLOTS], FP32, kind="ExternalInput")
    Wl_d = nc.dram_tensor("Wl", [L, 128, 128], FP32, kind="ExternalInput")
    Wr_d = nc.dram_tensor("Wr", [L, 128, 128], FP32, kind="ExternalInput")
    attR_d = nc.dram_tensor("attR", [L, 128, 128], FP32, kind="ExternalInput")
    xrb_d = nc.dram_tensor("xrb", [L, 128], FP32, kind="ExternalInput")
    beff_d = nc.dram_tensor("beff", [L, 64], FP32, kind="ExternalInput")
    g_d = nc.dram_tensor("gbn", [L, 64], FP32, kind="ExternalInput")
    be_d = nc.dram_tensor("bebn", [L, 64], FP32, kind="ExternalInput")
    ident_d = nc.dram_tensor("ident", [128, 128], FP32, kind="ExternalInput")
    headW_d = nc.dram_tensor("headW", [128, 2], FP32, kind="ExternalInput")
    headb_d = nc.dram_tensor("headb", [2, 1], FP32, kind="ExternalInput")

    imp_d = nc.dram_tensor("out_imp", [1, NPAD], FP32, kind="ExternalOutput")
    pol_d = nc.dram_tensor("out_pol", [1, NPAD], FP32, kind="ExternalOutput")
    dbg_d = None
    if cfg.dump_layer >= 0:
        dbg_d = nc.dram_tensor("out_dbg", [128, HALF2], FP32, kind="ExternalOutput")

    with tile.TileContext(nc) as tc:
        with (
            tc.tile_pool(name="persist", bufs=1) as P,
            tc.tile_pool(name="wload", bufs=2) as WP,
            tc.tile_pool(name="gpool", bufs=2) as GP,
            tc.tile_pool(name="spool", bufs=1) as SP,
            tc.tile_pool(name="small", bufs=3) as SM,
            tc.tile_pool(name="stage", bufs=2) as ST,
            tc.tile_pool(name="ochunk", bufs=2) as OC,
            tc.tile_pool(name="xstream", bufs=2) as XS,
            tc.tile_pool(name="mm_ps", bufs=4, space="PSUM") as MMP,
            tc.tile_pool(name="xr_ps", bufs=2, space="PSUM") as XRP,
            tc.tile_pool(name="tr_ps", bufs=2, space="PSUM") as TRP,
            tc.tile_pool(name="dram", bufs=2, space="DRAM") as DP,
        ):
            # ---------------- persistent SBUF ----------------
            h_fold = P.tile([128, HALF2], FP32, tag="h_fold")
            h_loc = P.tile([64, SLOTS], FP32, tag="h_loc")
            idx_sb = P.tile([128, 8 * SUMK], I16, tag="idx_sb")
            mask_sb = P.tile([128, SUMK], FP32, tag="mask_sb")
            ones_sb = P.tile([1, 128], FP32, tag="ones_sb")
            ident_sb = P.tile([128, 128], FP32, tag="ident_sb")
            o_slab = P.tile([64, SLOTS], FP32, tag="o_slab")
            xloc_sb = P.tile([32, SLOTS], FP32, tag="xloc_sb")

            nc.sync.dma_start(idx_sb[:], idx_d[:, :])
            nc.sync.dma_start(mask_sb[:], mask_d[:, :])
            nc.sync.dma_start(ident_sb[:], ident_d[:, :])
            nc.sync.dma_start(xloc_sb[:], xloc_d[:, :])
            nc.vector.memset(ones_sb[:], 1.0)
            if cfg.stage < 4:
                nc.vector.memset(h_fold[:], 0.0)
                nc.vector.memset(h_loc[:], 0.0)
                nc.vector.memset(o_slab[:], 0.0)

            koff = [0]
            for K in cfg.K_sched:
                koff.append(koff[-1] + int(K))

            for l in range(L):
                w = cfg.width(l)       # H*D of this layer
                w2 = w // 2            # per-head width = output width
                KC = cfg.kcontract(l)  # matmul contraction
                last = l == L - 1

                # -------- per-layer weight loads --------
                Wl_sb = WP.tile([128, 128], FP32, tag="Wl_sb")
                nc.sync.dma_start(Wl_sb[:], Wl_d.ap()[l : l + 1].squeeze(0))
                Wr_sb = WP.tile([128, 128], FP32, tag="Wr_sb")
                nc.sync.dma_start(Wr_sb[:], Wr_d.ap()[l : l + 1].squeeze(0))
                attR_sb = WP.tile([128, 128], FP32, tag="attR_sb")
                nc.sync.dma_start(attR_sb[:], attR_d.ap()[l : l + 1].squeeze(0))
                xrb_sb = WP.tile([1, 128], FP32, tag="xrb_sb")
                nc.sync.dma_start(xrb_sb[:], xrb_d.ap()[l : l + 1, :])
                beff_sb = WP.tile([64, 1], FP32, tag="beff_sb")
                nc.sync.dma_start(beff_sb[:], beff_d.ap()[l : l + 1, :].rearrange("o f -> f o"))
                g_sb = WP.tile([64, 1], FP32, tag="g_sb")
                nc.sync.dma_start(g_sb[:], g_d.ap()[l : l + 1, :].rearrange("o f -> f o"))
                be_sb = WP.tile([64, 1], FP32, tag="be_sb")
                nc.sync.dma_start(be_sb[:], be_d.ap()[l : l + 1, :].rearrange("o f -> f o"))

                # -------- xl table: [NPAD, 128-pitch] in DRAM --------
                xl_tab = DP.tile([NPAD, 128], FP32, tag="xl_tab")
                n_groups = HALF2 // 512
                for g in range(n_groups):
                    if l == 0:
                        xch = XS.tile([64, 512], FP32, tag="xch")
                        nc.sync.dma_start(
                            xch[:], xTf_d.ap()[:, g * 512 : g * 512 + 512]
                        )
                    for half in range(2):
                        stg = ST.tile([128, 4, 128], FP32, tag="stg")
                        for q in range(4):
                            if l == 0:
                                lhsT = xch[half * 32 : half * 32 + 32,
                                           q * 128 : q * 128 + 128]
                            else:
                                j = g * 4 + q
                                lhsT = h_fold[half * 64 : half * 64 + 64,
                                              j * 128 : j * 128 + 128]
                            ps = MMP.tile([128, 128], FP32, tag="mm")
                            nc.tensor.matmul(
                                ps[:, 0:w],
                                lhsT,
                                Wl_sb[half * KC : half * KC + KC, 0:w],
                                start=True, stop=True,
                            )
                            nc.scalar.copy(
                                stg[:, q : q + 1, 0:w].squeeze(1), ps[:, 0:w]
                            )
                        slot0 = half * HALF2 + g * 512
                        nc.sync.dma_start(
                            xl_tab[:]
                            .rearrange("(s p) c -> p s c", p=128)[
                                :, slot0 // 128 : slot0 // 128 + 4, 0:w
                            ],
                            stg[:, :, 0:w],
                        )

                # -------- per-block edge processing --------
                if cfg.stage < 2:
                    break
                for b in range(cfg.blocks):
                    K = int(cfg.K_sched[b])
                    # xr for this block: bias-seeded accumulating matmul
                    xr_ps = XRP.tile([128, 128], FP32, tag="xr")
                    nc.tensor.matmul(
                        xr_ps[:, 0:w], ones_sb[:], xrb_sb[:, 0:w],
                        start=True, stop=False,
                    )
                    loc = xloc_sb if l == 0 else h_loc
                    nc.tensor.matmul(
                        xr_ps[:, 0:w],
                        loc[0:KC, b * 128 : b * 128 + 128],
                        Wr_sb[0:KC, 0:w],
                        start=False, stop=True,
                    )
                    xr_sb = SM.tile([128, 128], FP32, tag="xr_sb")
                    nc.scalar.copy(xr_sb[:, 0:w], xr_ps[:, 0:w])
                    if cfg.bstage < 2:
                        continue

                    # gather xl[src] for the block's edge slots
                    if l not in cfg.gather_layers:
                        continue
                    gt = GP.tile([128, KMAX, w], FP32, tag="g")
                    nc.gpsimd.dma_gather(
                        gt[:, 0:K, :],
                        xl_tab[:, 0:w],
                        idx_sb[:, 8 * koff[b] : 8 * koff[b] + 8 * K],
                        128 * K, 128 * K, w, elem_step=128, single_packet=False,
                    )

                    if cfg.bstage < 3:
                        continue
                    # s = lrelu(g + xr) * att
                    s_t = SP.tile([128, KMAX, w], FP32, tag="s", name="s_t")
                    s = s_t[:, 0:K, :]
                    nc.vector.tensor_tensor(
                        s, gt[:, 0:K, :],
                        xr_sb[:, 0:w].unsqueeze(1).broadcast_to([128, K, w]),
                        ALU.add,
                    )
                    nc.scalar.activation(s, s, AF.Prelu, alpha=SLOPE)
                    nc.vector.tensor_tensor(
                        s, s,
                        attR_sb[:, 0:w].unsqueeze(1).broadcast_to([128, K, w]),
                        ALU.mult,
                    )

                    if cfg.bstage < 4:
                        continue
                    # logit[d, k, h] (+ mask)
                    lg_t = SM.tile([128, KMAX, 2], FP32, tag="lg", name="lg_t")
                    lg = lg_t[:, 0:K, :]
                    nc.vector.tensor_reduce(
                        lg, s.rearrange("p k (h c) -> p k h c", h=2), X, ALU.add,
                    )
                    nc.vector.tensor_tensor(
                        lg, lg,
                        mask_sb[:, koff[b] : koff[b] + K]
                        .unsqueeze(2).broadcast_to([128, K, 2]),
                        ALU.add,
                    )

                    if cfg.bstage < 5:
                        continue
                    # softmax over k per head (negated max -> exp bias)
                    nm = SM.tile([128, 2], FP32, tag="nm")
                    nc.vector.tensor_reduce(
                        nm[:], lg.transpose([0, 2, 1]), X, ALU.max, negate=True,
                    )
                    av_t = SM.tile([128, 2, KMAX], FP32, tag="av", name="av_t")
                    av = av_t[:, :, 0:K]
                    for h in range(2):
                        nc.scalar.activation(
                            av[:, h : h + 1, :].squeeze(1),
                            lg[:, :, h : h + 1].squeeze(2),
                            AF.Exp, bias=nm[:, h : h + 1],
                        )
                    ssum = SM.tile([128, 2], FP32, tag="ssum")
                    nc.vector.tensor_reduce(ssum[:], av, X, ALU.add)
                    # fold the head-mean 0.5 into the normalizer
                    nc.vector.tensor_scalar_mul(ssum[:], ssum[:], 2.0)
                    rec = SM.tile([128, 2], FP32, tag="rec")
                    nc.vector.reciprocal(rec[:], ssum[:])
                    alf_t = SM.tile([128, 2, KMAX], FP32, tag="alf", name="alf_t")
                    alf = alf_t[:, :, 0:K]
                    nc.vector.tensor_tensor(
                        alf, av,
                        rec[:].unsqueeze(2).broadcast_to([128, 2, K]),
                        ALU.mult,
                    )

                    if cfg.bstage < 6:
                        continue
                    # weighted sum: g *= alpha (bcast over c), tree-reduce over k
                    alf_b = (
                        alf.transpose([0, 2, 1])
                        .unsqueeze(3)
                        .broadcast_to([128, K, 2, w2])
                    )
                    g4 = gt[:, 0:K, :].rearrange("p k (h c) -> p k h c", h=2)
                    nc.vector.tensor_tensor(g4, g4, alf_b, ALU.mult)
                    cur = K
                    while cur > 1:
                        hlf = cur // 2
                        nc.vector.tensor_tensor(
                            gt[:, 0:hlf, :],
                            gt[:, 0:hlf, :],
                            gt[:, cur - hlf : cur, :],
                            ALU.add,
                        )
                        cur = cur - hlf
                    # head-mean (0.5 folded into alpha already)
                    ob_t = SM.tile([128, 64], FP32, tag="ob", name="ob_t")
                    ob = ob_t[:, 0:w2]
                    nc.vector.tensor_add(
                        ob,
                        gt[:, 0:1, 0:w2].squeeze(1),
                        gt[:, 0:1, w2:w].squeeze(1),
                    )
                    if cfg.bstage < 7:
                        continue
                    # transpose to feature-major and add bias_eff
                    tp = TRP.tile([64, 128], FP32, tag="tp")
                    nc.tensor.transpose(tp[0:w2, :], ob, ident_sb[:])
                    nc.scalar.activation(
                        o_slab[0:w2, b * 128 : b * 128 + 128],
                        tp[0:w2, :], AF.Identity, bias=beff_sb[0:w2, :],
                    )

                # zero dead columns of the slab
                if cfg.real_per_core < SLOTS:
                    nc.vector.memset(o_slab[0:w2, cfg.real_per_core : SLOTS], 0.0)

                # -------- AllGather of the pre-BN slab --------
                if cfg.stage < 3:
                    break
                agtag = "ag_in" if w2 == 64 else "ag_in_l"
                ag_in = DP.tile([1, w2 * SLOTS], FP32, tag=agtag,
                                bufs=2 if w2 == 64 else 1)
                nc.sync.dma_start(
                    ag_in[:, :].rearrange("o (p f) -> (o p) f", p=w2),
                    o_slab[0:w2, :],
                )
                agotag = "ag_out" if w2 == 64 else "ag_out_l"
                ag_out = DP.tile([NC, w2 * SLOTS], FP32, tag=agotag,
                                 addr_space="Shared",
                                 bufs=2 if w2 == 64 else 1)
                nc.gpsimd.collective_compute(
                    "AllGather",
                    ALU.bypass,
                    ins=[ag_in.opt()],
                    outs=[ag_out.opt()],
                    replica_groups=[list(range(NC))],
                )

                # -------- global BN stats --------
                nch = NC // 2
                ranges = [(0, 128)] if w2 == 64 else [(0, 32), (64, 96)]
                sums = SM.tile([128, 4], FP32, tag="sums")
                sqs = SM.tile([128, 4], FP32, tag="sqs")
                for c4 in range(nch):
                    och = OC.tile([128, SLOTS], FP32, tag="och")
                    for hi in range(2):
                        nc.sync.dma_start(
                            och[64 * hi : 64 * hi + w2, :],
                            ag_out[hi * nch + c4 : hi * nch + c4 + 1, :]
                            .rearrange("o (p f) -> (o p) f", p=w2),
                        )
                    scratch = OC.tile([128, SLOTS], FP32, tag="bigs")
                    for (p0, p1) in ranges:
                        nc.vector.tensor_reduce(
                            sums[p0:p1, c4 : c4 + 1], och[p0:p1, :], X, ALU.add,
                        )
                        nc.scalar.activation(
                            scratch[p0:p1, :], och[p0:p1, :], AF.Square,
                            accum_out=sqs[p0:p1, c4 : c4 + 1],
                        )
                s128 = SM.tile([128, 2], FP32, tag="s128")
                for col, acc in ((0, sums), (1, sqs)):
                    for (p0, p1) in ranges:
                        nc.vector.tensor_reduce(
                            s128[p0:p1, col : col + 1], acc[p0:p1, 0:nch], X, ALU.add,
                        )
                # combine fold halves across partitions via SBUF->SBUF DMA
                s64 = SM.tile([64, 2], FP32, tag="s64")
                nc.sync.dma_start(s64[0:w2, :], s128[64 : 64 + w2, :])
                nc.vector.tensor_add(s64[0:w2, :], s64[0:w2, :], s128[0:w2, :])

                # mu, var, scale, bias (on partitions 0:w2)
                stat = SM.tile([64, 4], FP32, tag="stat")
                nc.vector.tensor_scalar_mul(
                    stat[0:w2, 0:2], s64[0:w2, :], 1.0 / N_REAL
                )
                mu = stat[0:w2, 0:1]
                msq = stat[0:w2, 1:2]
                var = stat[0:w2, 2:3]
                nc.vector.tensor_tensor(var, mu, mu, ALU.mult)
                nc.vector.tensor_sub(var, msq, var)
                # rstd = exp(-0.5 * ln(var + eps))
                lnv = stat[0:w2, 3:4]
                nc.vector.tensor_scalar_add(var, var, float(EPS))
                nc.scalar.activation(lnv, var, AF.Ln)
                sc = SM.tile([128, 2], FP32, tag="sc")
                nc.scalar.activation(sc[0:w2, 0:1], lnv, AF.Exp, scale=-0.5)
                # scale = g * rstd ; bias = be - mu * scale
                nc.vector.tensor_tensor(
                    sc[0:w2, 0:1], sc[0:w2, 0:1], g_sb[0:w2, :], ALU.mult
                )
                nc.vector.tensor_tensor(sc[0:w2, 1:2], mu, sc[0:w2, 0:1], ALU.mult)
                nc.vector.tensor_sub(sc[0:w2, 1:2], be_sb[0:w2, :], sc[0:w2, 1:2])
                # replicate to fold partitions 64:64+w2
                nc.sync.dma_start(sc[64 : 64 + w2, :], sc[0:w2, :])

                # -------- h update (folded, all cores' columns) --------
                for c4 in range(nch):
                    och2 = OC.tile([128, SLOTS], FP32, tag="och")
                    for hi in range(2):
                        nc.sync.dma_start(
                            och2[64 * hi : 64 * hi + w2, :],
                            ag_out[hi * nch + c4 : hi * nch + c4 + 1, :]
                            .rearrange("o (p f) -> (o p) f", p=w2),
                        )
                    bn = OC.tile([128, SLOTS], FP32, tag="bigs")
                    for (p0, p1) in ranges:
                        nc.scalar.activation(
                            bn[p0:p1, :], och2[p0:p1, :], AF.Relu,
                            scale=sc[p0:p1, 0:1], bias=sc[p0:p1, 1:2],
                        )
                        dst = h_fold[p0:p1, c4 * SLOTS : (c4 + 1) * SLOTS]
                        if l == 0 or last:
                            nc.vector.tensor_copy(dst, bn[p0:p1, :])
                        else:
                            nc.vector.tensor_tensor(dst, dst, bn[p0:p1, :], ALU.add)

                if cfg.dump_layer == l:
                    nc.sync.dma_start(dbg_d[:, :], h_fold[:, :])
                    break

                # -------- h_loc update (from local slab, no collective) ----
                if not last:
                    bnl = OC.tile([128, SLOTS], FP32, tag="bigs")
                    nc.scalar.activation(
                        bnl[0:64, :], o_slab[0:64, :], AF.Relu,
                        scale=sc[0:64, 0:1], bias=sc[0:64, 1:2],
                    )
                    if l == 0:
                        nc.vector.tensor_copy(h_loc[:], bnl[0:64, :])
                    else:
                        nc.vector.tensor_tensor(
                            h_loc[:], h_loc[:], bnl[0:64, :], ALU.add
                        )

            # ---------------- output heads ----------------
            if cfg.stage < 4:
                z = P.tile([2, 512], FP32, tag="zz")
                nc.vector.memset(z[:], 0.0)
                for half in range(2):
                    for j in range(HALF2 // 512):
                        col0 = half * HALF2 + j * 512
                        nc.sync.dma_start(imp_d.ap()[:, col0 : col0 + 512], z[0:1, :])
                        nc.sync.dma_start(pol_d.ap()[:, col0 : col0 + 512], z[1:2, :])
            headW_sb = P.tile([128, 2], FP32, tag="headW_sb")
            if cfg.stage < 4:
                headW_sb = None
            if headW_sb is not None:
                nc.sync.dma_start(headW_sb[:], headW_d[:, :])
            headb_sb = P.tile([2, 1], FP32, tag="headb_sb")
            if headW_sb is None:
                headb_sb = None
            if headb_sb is not None:
                nc.sync.dma_start(headb_sb[:], headb_d[:, :])
            for half in range(2 if headW_sb is not None else 0):
                base = 64 * half
                for j in range(HALF2 // 512):
                    hp = TRP.tile([2, 512], FP32, tag="tp")
                    nc.tensor.matmul(
                        hp[:],
                        headW_sb[base : base + 32, :],
                        h_fold[base : base + 32, j * 512 : (j + 1) * 512],
                        start=True, stop=True,
                    )
                    hs = SM.tile([2, 512], FP32, tag="hs")
                    nc.scalar.activation(hs[:], hp[:], AF.Identity, bias=headb_sb[:])
                    hs2 = SM.tile([2, 512], FP32, tag="hs2")
                    nc.scalar.activation(hs2[:], hs[:], AF.Sigmoid)
                    col0 = half * HALF2 + j * 512
                    nc.sync.dma_start(imp_d.ap()[:, col0 : col0 + 512], hs[0:1, :])
                    nc.sync.dma_start(pol_d.ap()[:, col0 : col0 + 512], hs2[1:2, :])

    nc.compile()
    return nc


# ===================== host side =====================

def make_cfg(deg, ncores=8, nlayers=20, f_in=17):
    n = deg.shape[0]
    real = n // ncores
    blocks = (real + 127) // 128
    order = np.argsort(deg, kind="stable")
    Ks = np.zeros((ncores, blocks), np.int64)
    for c in range(ncores):
        dc = deg[order[c::ncores]]
        for b in range(blocks):
            blk = dc[b * 128 : (b + 1) * 128]
            Ks[c, b] = blk.max() if blk.size else 1
    K_sched = tuple(int(max(x, 1)) for x in Ks.max(axis=0))
    cfg = Cfg(ncores=ncores, blocks=blocks, real_per_core=real,
              nlayers=nlayers, K_sched=K_sched, f_in=f_in)
    return order, cfg


def host_prep(inputs, nlayers=20, ncores=8):
    """Build cfg, per-core input maps, and the slot->node mapping."""
    x = np.asarray(inputs["x"], np.float32)
    src = np.asarray(inputs["src"], np.int64)
    dst = np.asarray(inputs["dst"], np.int64)
    n = x.shape[0]
    loop = np.arange(n, dtype=np.int64)
    s_all = np.concatenate([src, loop])
    d_all = np.concatenate([dst, loop])
    deg = np.bincount(d_all, minlength=n)

    order, cfg = make_cfg(deg, ncores=ncores, nlayers=nlayers, f_in=x.shape[1])
    SLOTS = cfg.slots
    NPAD = cfg.npad
    L = nlayers

    slot_of_node = np.full(n, -1, np.int64)
    for c in range(ncores):
        nodes = order[c::ncores]
        slot_of_node[nodes] = c * SLOTS + np.arange(nodes.shape[0])
    assert (slot_of_node >= 0).all()

    s_slot = slot_of_node[s_all]
    d_slot = slot_of_node[d_all]

    Ksch = cfg.K_sched
    sumk = cfg.sumk
    koff = np.concatenate([[0], np.cumsum(Ksch)]).astype(np.int64)
    order_e = np.argsort(d_slot, kind="stable")
    ds_sorted = d_slot[order_e]
    ss_sorted = s_slot[order_e]
    starts = np.searchsorted(ds_sorted, np.arange(NPAD))
    ends = np.searchsorted(ds_sorted, np.arange(NPAD) + 1)

    idx_maps, mask_maps = [], []
    for c in range(ncores):
        mask = np.full((128, sumk), np.float32(NEG), np.float32)
        idx_arr = np.zeros((128, 8 * sumk), np.int16)
        for b in range(cfg.blocks):
            K = int(Ksch[b])
            flat = np.zeros(128 * K, np.int16)
            for lane in range(128):
                sl = c * SLOTS + b * 128 + lane
                e0, e1 = starts[sl], ends[sl]
                kdeg = e1 - e0
                assert kdeg <= K, (kdeg, K, b)
                if kdeg:
                    flat[np.arange(kdeg) * 128 + lane] = ss_sorted[e0:e1].astype(np.int16)
                    mask[lane, koff[b] : koff[b] + kdeg] = 0.0
            blk = flat.reshape(8 * K, 16)
            idx_arr[:, 8 * koff[b] : 8 * koff[b] + 8 * K] = np.tile(blk.T, (8, 1))
        idx_maps.append(idx_arr)
        mask_maps.append(mask)

    xp = np.zeros((NPAD, 32), np.float32)
    xp[slot_of_node[np.arange(n)], : x.shape[1]] = x
    HALF2 = cfg.half2
    xTf = np.ascontiguousarray(
        np.concatenate([xp[:HALF2].T, xp[HALF2:].T], axis=0)[:, :]
    ).astype(np.float32)
    xloc_maps = [
        np.ascontiguousarray(xp[c * SLOTS : (c + 1) * SLOTS].T) for c in range(ncores)
    ]

    def wpack(W, KC):
        out = np.zeros((128, 128), np.float32)
        kin, wout = W.shape
        out[0:kin, 0:wout] = W
        out[KC : KC + kin, 0:wout] = W
        return out

    Wl_all = np.zeros((L, 128, 128), np.float32)
    Wr_all = np.zeros((L, 128, 128), np.float32)
    attR = np.zeros((L, 128, 128), np.float32)
    xrb = np.zeros((L, 128), np.float32)
    beff = np.zeros((L, 64), np.float32)
    g_all = np.zeros((L, 64), np.float32)
    be_all = np.zeros((L, 64), np.float32)

    def layer_params(l):
        if l == 0:
            return (inputs["W0l"], inputs["b0l"], inputs["W0r"], inputs["b0r"],
                    inputs["att0"], inputs["bias0"], inputs["g0"], inputs["be0"])
        if l < L - 1:
            i = l - 1
            return (inputs["Wml"][i], inputs["bml"][i], inputs["Wmr"][i],
                    inputs["bmr"][i], inputs["attm"][i], inputs["biasm"][i],
                    inputs["gm"][i], inputs["bem"][i])
        return (inputs["WLl"], inputs["bLl"], inputs["WLr"], inputs["bLr"],
                inputs["attL"], inputs["biasL"], inputs["gL"], inputs["beL"])

    for l in range(L):
        Wl, bl, Wr, br, att, bias, g, be = [
            np.asarray(a, np.float32) for a in layer_params(l)
        ]
        KC = 32 if l == 0 else 64
        w = 128 if l < L - 1 else 64
        w2 = w // 2
        Wl_all[l] = wpack(Wl, KC)
        Wr_all[l] = wpack(Wr, KC)
        attR[l, :, 0:w] = np.tile(att.reshape(1, w), (128, 1))
        xrb[l, 0:w] = bl + br
        beff[l, 0:w2] = bias + 0.5 * (bl.reshape(2, w2)[0] + bl.reshape(2, w2)[1])
        g_all[l, 0:w2] = g
        be_all[l, 0:w2] = be

    headW = np.zeros((128, 2), np.float32)
    Wimp = np.asarray(inputs["Wimp"], np.float32)
    Wpol = np.asarray(inputs["Wpol"], np.float32)
    d_out = Wimp.shape[0]
    headW[0:d_out, 0] = Wimp[:, 0]
    headW[0:d_out, 1] = Wpol[:, 0]
    headW[64 : 64 + d_out, 0] = Wimp[:, 0]
    headW[64 : 64 + d_out, 1] = Wpol[:, 0]
    headb = np.array(
        [[np.float32(np.asarray(inputs["bimp"]).reshape(-1)[0])],
         [np.float32(np.asarray(inputs["bpol"]).reshape(-1)[0])]], np.float32
    )

    shared = dict(
        xTf=xTf,
        Wl=Wl_all, Wr=Wr_all, attR=attR, xrb=xrb, beff=beff,
        gbn=g_all, bebn=be_all,
        ident=np.eye(128, dtype=np.float32),
        headW=headW, headb=headb,
    )
    in_maps = []
    for c in range(ncores):
        m = dict(shared)
        m["idx"] = idx_maps[c]
        m["mask"] = mask_maps[c]
        m["xloc"] = xloc_maps[c]
        in_maps.append(m)
    return cfg, in_maps, slot_of_node


# ===================== entry point =====================

LAST_EXEC_NS = None
_TRACE = bool(int(__import__("os").environ.get("GNN_TRACE", "0")))


def kernel(**inputs):
    """Full-input GATv2 GNN on 8 trn2 NeuronCores; returns (imp, pol)."""
    global LAST_EXEC_NS
    from concourse.bass_utils import run_bass_kernel_spmd

    cfg, in_maps, slot_of_node = host_prep(inputs, nlayers=20, ncores=8)
    nc = build_kernel(cfg)
    res = run_bass_kernel_spmd(
        nc, in_maps, core_ids=list(range(cfg.ncores)), trace=_TRACE
    )
    LAST_EXEC_NS = res.exec_time_ns
    imp = np.asarray(res.results[0]["out_imp"]).reshape(-1)[slot_of_node][:, None]
    pol = np.asarray(res.results[0]["out_pol"]).reshape(-1)[slot_of_node][:, None]
    return imp.astype(np.float32), pol.astype(np.float32)

